# revision 29
# baseline (speedup 1.0000x reference)
"""Bass/Tile TRN2 kernel for nn_DifferentialWordSegmentation.

kernel(**inputs) takes the FULL unsharded inputs (numpy), shards batch B=32
across 8 NeuronCores (4 rows each, pure data parallel), runs one SPMD Bass
kernel, and returns the full (32, 1024, 512) float32 output.

Self-contained: shapes/sharding hardcoded, no sibling imports.

The segment-similarity MLP (the dominant matmul) runs as a 3-term
compensated product in fp16/bf16 (main pieces fp16, residual pieces bf16;
1 PE cycle/row instead of 4 for fp32) — the realized S error is ~5e-7 in
peak-detector units vs a minimum decision margin of 2.5e-5, so every
boundary decision matches the fp32 reference. The word-pooling/MLP stage
runs in fp16 (same 11-bit mantissa as the f32r path it replaces).
"""
import os
import numpy as np

import concourse.bacc as bacc
import concourse.mybir as mybir
import concourse.tile as tile
from concourse.bass_utils import run_bass_kernel_spmd

F32 = mybir.dt.float32
F16 = mybir.dt.float16
BF16 = mybir.dt.bfloat16
AF = mybir.ActivationFunctionType
OP = mybir.AluOpType

B, N, H = 32, 1024, 512
NCORES = 8
RPC = B // NCORES          # rows per core = 4
GROUPS = ((0, 1, 2), (3,))  # phase-C row groups (C(g0) hides under stage1(3))
NT = N // 128              # 8 i-tiles
HT = H // 128              # 4 h-tiles
THR = 0.05
SEC = N + 8                # xhT/xlT section stride (pad for +1 shifts)
MS = 384                   # static word range [0, MS); rest under runtime If

DEBUG = bool(int(os.environ.get("KERNEL_DEBUG", "0")))
MASK_ONES = False   # set by kernel() when phn_mask is all ones (specialized build)
BIAS0 = False       # set by kernel() when be2 is all zeros (specialized build)
SIM_SKIP = bool(int(os.environ.get("KERNEL_SIM_SKIP", "0")))

_cached = {}


def _build_module():
    nc = bacc.Bacc(trn_type="TRN2", target_bir_lowering=False, debug=False)

    x_d = nc.dram_tensor("x", [RPC, N, H], F32, kind="ExternalInput").ap()
    mask_d = nc.dram_tensor("mask", [RPC, N], F32, kind="ExternalInput").ap()
    W1_d = nc.dram_tensor("W1", [2 * H, H], F32, kind="ExternalInput").ap()
    b1_d = nc.dram_tensor("b1", [H], F32, kind="ExternalInput").ap()
    W2_d = nc.dram_tensor("W2", [H, 1], F32, kind="ExternalInput").ap()
    We1_d = nc.dram_tensor("We1", [H, H], F32, kind="ExternalInput").ap()
    be1_d = nc.dram_tensor("be1", [H], F32, kind="ExternalInput").ap()
    We2_d = nc.dram_tensor("We2", [H, H], F32, kind="ExternalInput").ap()
    be2_d = nc.dram_tensor("be2", [H], F32, kind="ExternalInput").ap()
    iota_d = nc.dram_tensor("iota1024", [1, N], F32, kind="ExternalInput").ap()
    i128_d = nc.dram_tensor("iota128", [1, 128], F32, kind="ExternalInput").ap()
    idx_d = nc.dram_tensor("idx128", [128, 1], F32, kind="ExternalInput").ap()
    out_d = nc.dram_tensor("out", [RPC, N, H], F32, kind="ExternalOutput").ap()
    dumps = {}
    if DEBUG:
        for nm, shp in (("S_dump", [RPC, N]), ("P_dump", [RPC, N]),
                        ("b_dump", [RPC, N]), ("c_dump", [RPC, N])):
            dumps[nm] = nc.dram_tensor(nm, shp, F32, kind="ExternalOutput").ap()

    with tile.TileContext(nc) as tc:
        _emit(nc, tc, x_d, mask_d, W1_d, b1_d, W2_d, We1_d, be1_d, We2_d,
              be2_d, iota_d, i128_d, idx_d, out_d, dumps)
    nc.compile()
    return nc


def _emit(nc, tc, x_d, mask_d, W1_d, b1_d, W2_d, We1_d, be1_d, We2_d, be2_d,
          iota_d, i128_d, idx_d, out_d, dumps):
    from contextlib import ExitStack
    ctx = ExitStack()
    pool = lambda name, bufs, **kw: ctx.enter_context(
        tc.tile_pool(name=name, bufs=bufs, **kw))

    const = pool("const", 1)
    wpool = pool("weights", 1)
    xn_p = pool("xn", 1)       # raw x (128, 512) f32, tags xn0..7
    xsc_p = pool("xsc", 1)     # normalized x f32, tags xs0..3 (half-row live)
    xt_p = pool("xt", 2)       # xhT fp16 / xlT bf16 big tiles per row
    rel_p = pool("rel", 1)     # rh fp16 / rl bf16 big tiles per row
    big_a = pool("biga", 1)    # stage3 wr (128, 512) fp16, tags bg0..3
    big_b = pool("bigb", 1)    # stage3 r1m (128, 512) fp16, tags bb0..3
    x16_p = pool("x16", 1)     # stage3 x fp16 tiles, tags xr0..7
    sg_p = pool("sg", 1)       # stage3 sg fp16 tiles, tags sg0..7
    scr = pool("scratch", 2)
    tiny = pool("tiny", 2)
    cpool = pool("phasec", 1)
    outp = pool("outstage", 2)
    psA = pool("psA", 3, space="PSUM")
    psB = pool("psB", 4, space="PSUM")
    psS = pool("psS", 1, space="PSUM")

    # ---- tiny constants first: ident128 gates the first transposes ----
    i128_bc = const.tile([128, 128], F32, name="i128_bc")   # rows of 0..127
    nc.sync.dma_start(i128_bc[:], i128_d.to_broadcast((128, 128)))
    idxcol = const.tile([128, 1], F32, name="idxcol")       # 0..127
    nc.sync.dma_start(idxcol[:], idx_d)
    # ---- prefetch row 0's x tiles (SP ring) so PE can start early ----
    xpre = [xn_p.tile([128, H], F32, name=f"xn_0_{t}", tag=f"xn{t}")
            for t in range(NT)]
    for t in range(NT):
        nc.sync.dma_start(xpre[t][:], x_d[0, t * 128:(t + 1) * 128, :])
    iota_bc = const.tile([128, N], F32, name="iota_bc")     # rows of 1..1024
    nc.sync.dma_start(iota_bc[:], iota_d.to_broadcast((128, N)))
    be2_bc = const.tile([128, H], F32, name="be2_bc")
    nc.sync.dma_start(be2_bc[:],
                        be2_d.rearrange("(o h) -> o h", o=1).to_broadcast((128, H)))
    ident128 = const.tile([128, 128], F32, name="ident128")
    nc.vector.tensor_scalar(ident128[:], i128_bc[:, 0:128], idxcol[:], None,
                            op0=OP.is_equal)
    zeros_bc = const.tile([3, N], F32, name="zeros_bc")
    nc.vector.memset(zeros_bc[:], 0.0)
    ones16 = const.tile([128, 1], F16, name="ones16")
    nc.vector.tensor_scalar(ones16[:], idxcol[:], -1.0, None, op0=OP.is_gt)
    ident4 = const.tile([4, 4], F32, name="ident4")
    nc.vector.tensor_scalar(ident4[:], i128_bc[0:4, 0:4], idxcol[0:4, :], None,
                            op0=OP.is_equal)

    # ---- weights: W1 -> fp16 high + bf16 residual (3-term scheme) ----
    Wh = [wpool.tile([128, H], F16, name=f"wh_{k}") for k in range(2 * HT)]
    Wl = [wpool.tile([128, H], BF16, name=f"wl_{k}") for k in range(2 * HT)]
    We1h = [wpool.tile([128, H], F16, name=f"we1h_{k}") for k in range(HT)]
    We2h = [wpool.tile([128, H], F16, name=f"we2h_{k}") for k in range(HT)]
    for k in range(2 * HT):
        wtmp = scr.tile([128, H], F32, name=f"wtmp_{k}", tag="ut")
        nc.sync.dma_start(wtmp[:], W1_d[k * 128:(k + 1) * 128, :])
        nc.gpsimd.tensor_copy(Wh[k][:], wtmp[:])
        nc.vector.tensor_tensor(Wl[k][:], wtmp[:], Wh[k][:], op=OP.subtract)
    for k in range(HT):
        wtmp1 = scr.tile([128, H], F32, name="wtmpe1", tag="ut")
        nc.sync.dma_start(wtmp1[:], We1_d[k * 128:(k + 1) * 128, :])
        nc.gpsimd.tensor_copy(We1h[k][:], wtmp1[:])
        wtmp2 = scr.tile([128, H], F32, name="wtmpe2", tag="ut")
        nc.sync.dma_start(wtmp2[:], We2_d[k * 128:(k + 1) * 128, :])
        nc.gpsimd.tensor_copy(We2h[k][:], wtmp2[:])
    w2c = wpool.tile([128, HT], F32, name="w2c")
    w2h = wpool.tile([128, HT], F16, name="w2h")
    w2l = wpool.tile([128, HT], BF16, name="w2l")
    w2_v = W2_d.rearrange("(k p) o -> k p o", p=128)
    b1c = wpool.tile([128, HT], F32, name="b1c")
    b1_v = b1_d.rearrange("(k p) -> k p", p=128)
    be1c = wpool.tile([128, HT], F32, name="be1c")
    be1_v = be1_d.rearrange("(k p) -> k p", p=128)
    for k in range(HT):
        nc.sync.dma_start(w2c[:, k:k + 1], w2_v[k])
        nc.sync.dma_start(b1c[:, k:k + 1], b1_v[k].unsqueeze(1))
        nc.sync.dma_start(be1c[:, k:k + 1], be1_v[k].unsqueeze(1))
    nc.vector.tensor_copy(w2h[:], w2c[:])
    nc.vector.tensor_tensor(w2l[:], w2c[:], w2h[:], op=OP.subtract)

    NG = len(GROUPS)
    clast_row = const.tile([1, RPC], F32, name="clast_row")
    Srow_g = [cpool.tile([3, N], F32, name=f"Srow_{g}", tag="srow")
              for g in range(NG)]
    mask_g = [const.tile([3, N], F32, name=f"mask_{g}") for g in range(NG)]
    if not MASK_ONES:
        for g, rows in enumerate(GROUPS):
            gn = len(rows)
            nc.sync.dma_start(mask_g[g][0:gn, :], mask_d[rows[0]:rows[0] + gn, :])
            nc.vector.tensor_scalar(mask_g[g][0:gn, :], mask_g[g][0:gn, :], 1.0,
                                    None, op0=OP.subtract)
    ct = cpool.tile([128, NT * RPC], F32, name="ct")

    # ------------- stage 1 per row: norms, transpose, split, G, S -------------
    def stage1(r, pre_w2=None):
        # xhT: fp16 [128, 4*SEC]; xlT: bf16 [128, 4*SEC]; k-major sections
        xhT = xt_p.tile([128, HT * SEC], F16, name=f"xhT_{r}", tag="xhT")
        xlT = xt_p.tile([128, HT * SEC], BF16, name=f"xlT_{r}", tag="xlT")
        for k in range(HT):
            nc.vector.memset(xhT[:, k * SEC + N:(k + 1) * SEC], 0.0)
            nc.vector.memset(xlT[:, k * SEC + N:(k + 1) * SEC], 0.0)

        def norm_tile(t):
            if r == 0:
                xnat = xpre[t]
            else:
                xnat = xn_p.tile([128, H], F32, name=f"xn_{r}_{t}", tag=f"xn{t}")
                nc.sync.dma_start(xnat[:], x_d[r, t * 128:(t + 1) * 128, :])
            sqs = scr.tile([128, H], F32, name="sqs", tag="sqs")
            ssq = tiny.tile([128, 1], F32, name=f"ssq_{r}_{t}", tag="ssq")
            nc.scalar.activation(sqs[:], xnat[:], AF.Square, accum_out=ssq[:])
            rno = tiny.tile([128, 1], F32, name=f"rno_{r}_{t}", tag="rno")
            nc.scalar.activation(rno[:], ssq[:], AF.Sqrt)
            rn = tiny.tile([128, 1], F32, name=f"rn_{r}_{t}", tag="rn")
            nc.vector.reciprocal(rn[:], rno[:])
            xsc = xsc_p.tile([128, H], F32, name=f"xsc_{r}_{t}", tag=f"xs{t % 4}")
            nc.scalar.mul(xsc[:], xnat[:], rn[:])
            return xsc

        for th in range(2):
            xscs = [norm_tile(th * 4 + tt) for tt in range(4)]
            # one PSUM bank per k holds the 4 transposed n-tiles of this half;
            # the 4 transposes share one accumulation group (disjoint columns)
            for k in range(HT):
                psT = psA.tile([128, 512], F32, name="psT", tag="pst")
                for tt in range(4):
                    nc.tensor.matmul(psT[:, tt * 128:(tt + 1) * 128],
                                     xscs[tt][:, k * 128:(k + 1) * 128],
                                     ident128[:], is_transpose=True,
                                     start=(tt == 0), stop=(tt == 3))
                sec = k * SEC + th * 512
                nc.scalar.activation(xhT[:, sec:sec + 512], psT[:], AF.Copy)
                nc.vector.tensor_tensor(xlT[:, sec:sec + 512], psT[:],
                                        xhT[:, sec:sec + 512], op=OP.subtract)

        # G = W1a^T xn[i] + W1b^T xn[i+1]: 3-term compensated fp16
        rh = rel_p.tile([128, HT * N], F16, name=f"rh_{r}", tag="rh")
        rl = rel_p.tile([128, HT * N], BF16, name=f"rl_{r}", tag="rl")
        for c in range(2):
            for j in range(HT):
                psg = psB.tile([128, 512], F32, name="psg", tag="mm")
                first = True
                for mov, Wset, last in ((xhT, Wh, False), (xhT, Wl, False),
                                        (xlT, Wh, True)):
                    for k in range(HT):
                        nc.tensor.matmul(
                            psg[:], Wset[k][:, j * 128:(j + 1) * 128],
                            mov[:, k * SEC + c * 512:k * SEC + c * 512 + 512],
                            start=first, stop=False)
                        first = False
                    for k in range(HT):
                        nc.tensor.matmul(
                            psg[:], Wset[HT + k][:, j * 128:(j + 1) * 128],
                            mov[:, k * SEC + c * 512 + 1:k * SEC + c * 512 + 513],
                            start=False, stop=(last and k == HT - 1))
                sec = j * N + c * 512
                nc.scalar.activation(rh[:, sec:sec + 512], psg[:], AF.Relu,
                                     bias=b1c[:, j:j + 1])
                r32 = scr.tile([128, 512], F32, name="r32", tag="r32")
                nc.scalar.activation(r32[:], psg[:], AF.Relu,
                                     bias=b1c[:, j:j + 1])
                nc.vector.tensor_tensor(rl[:, sec:sec + 512], r32[:],
                                        rh[:, sec:sec + 512], op=OP.subtract)
        if pre_w2 is not None:
            # slot cheap PE work (e.g. the phase-C ct transposes) here so its
            # results are ready before the next stage needs them
            pre_w2()
        # W2 groups emitted after BOTH chunks' psg matmuls: by the time the
        # in-order PE queue reaches them, rh/rl are ready (no head-of-line
        # stall on the ACT/DVE relu chain)
        for c in range(2):
            pss = psS.tile([1, 512], F32, name="pss", tag="pss")
            first = True
            for wstat, mov, last in ((w2h, rh, False), (w2h, rl, False),
                                     (w2l, rh, True)):
                for k in range(HT):
                    nc.tensor.matmul(pss[:], wstat[:, k:k + 1],
                                     mov[:, k * N + c * 512:k * N + c * 512 + 512],
                                     start=first, stop=(last and k == HT - 1))
                    first = False
            stmp = tiny.tile([1, 512], F32, name="stmp", tag="stmp")
            nc.vector.tensor_copy(stmp[:], pss[:])
            g = next(i for i, rows in enumerate(GROUPS) if r in rows)
            rr = GROUPS[g].index(r)
            nc.sync.dma_start(Srow_g[g][rr:rr + 1, c * 512:(c + 1) * 512], stmp[:])

    # ------------- stage 2: phase C on one (GR, N) row group -------------
    NV = N - 1  # 1023 valid S columns
    cc_holder = {}

    def phase_c(g):
        GN = len(GROUPS[g])
        r0 = GROUPS[g][0]
        Srow = Srow_g[g][0:GN]
        Smax = cpool.tile([3, 1], F32, name=f"Smax_{g}", tag="smax")[0:GN]
        Smin = cpool.tile([3, 1], F32, name=f"Smin_{g}", tag="smin")[0:GN]
        nc.vector.tensor_reduce(Smax[:], Srow[:, 0:NV], axis=mybir.AxisListType.X,
                                op=OP.max)
        nc.vector.tensor_reduce(Smin[:], Srow[:, 0:NV], axis=mybir.AxisListType.X,
                                op=OP.min)
        nrng = cpool.tile([3, 1], F32, name=f"nrng_{g}", tag="nrng")[0:GN]
        nc.vector.tensor_tensor(nrng[:], Smin[:], Smax[:], op=OP.subtract)
        nrinv = cpool.tile([3, 1], F32, name=f"nrinv_{g}", tag="nrinv")[0:GN]
        nc.vector.reciprocal(nrinv[:], nrng[:])
        if dumps:
            nc.sync.dma_start(dumps["S_dump"][r0:r0 + GN, :], Srow[:])
        D = Srow
        nc.vector.tensor_scalar(D[:], Srow[:], Smax[:], nrinv[:],
                                op0=OP.subtract, op1=OP.mult)

        fo = cpool.tile([3, N], F32, name=f"fo_{g}", tag="fo")[0:GN]
        so = cpool.tile([3, N], F32, name=f"so_{g}", tag="so")[0:GN]
        # only so[:, 1021:1023] is consumed without being written below
        # (fo covers [0:1023]; col 1023 of fo/so is never read)
        nc.vector.memset(so[:, 1021:1023], 0.0)
        ta = cpool.tile([3, N], F32, name=f"ta_{g}", tag="ta")[0:GN]
        tb = cpool.tile([3, N], F32, name=f"tb_{g}", tag="tb")[0:GN]
        L = 1020   # fo interior i = 1..1020 (1021/1022 use the overwrite formula)
        # min(D1-D0, D1-D2) = D1 - max(D0, D2)
        nc.vector.tensor_tensor(ta[:, 0:L], D[:, 0:L], D[:, 2:2 + L], op=OP.max)
        nc.vector.tensor_tensor(tb[:, 0:L], D[:, 1:1 + L], ta[:, 0:L],
                                op=OP.subtract)
        nc.vector.tensor_scalar(fo[:, 1:1 + L], tb[:, 0:L], 0.0, None, op0=OP.max)
        nc.vector.tensor_tensor(ta[:, 0:1], D[:, 0:1], D[:, 1:2], op=OP.subtract)
        nc.vector.tensor_scalar(fo[:, 0:1], ta[:, 0:1], 0.0, None, op0=OP.max)
        nc.vector.tensor_tensor(ta[:, 0:2], D[:, 1021:1023], D[:, 1019:1021],
                                op=OP.subtract)
        nc.vector.tensor_scalar(fo[:, 1021:1023], ta[:, 0:2], 0.0, None, op0=OP.max)
        L2 = 1019  # so interior i = 2..1020
        nc.vector.tensor_tensor(ta[:, 0:L2], D[:, 0:L2], D[:, 4:4 + L2], op=OP.max)
        nc.vector.tensor_tensor(tb[:, 0:L2], D[:, 2:2 + L2], ta[:, 0:L2],
                                op=OP.subtract)
        nc.vector.tensor_scalar(so[:, 2:2 + L2], tb[:, 0:L2], 0.0, None,
                                op0=OP.max)
        nc.vector.tensor_tensor(ta[:, 0:2], D[:, 0:2], D[:, 2:4], op=OP.subtract)
        nc.vector.tensor_scalar(so[:, 0:2], ta[:, 0:2], 0.0, None, op0=OP.max)

        P = cpool.tile([3, N], F32, name=f"P_{g}", tag="P")[0:GN]
        nc.vector.memset(P[:, NV:N], 0.0)   # pad col only; [0:NV] fully written
        nc.vector.tensor_tensor(ta[:, 0:NV], fo[:, 0:NV], so[:, 0:NV], op=OP.max)
        nc.vector.tensor_scalar(ta[:, 0:NV], ta[:, 0:NV], THR, 0.0,
                                op0=OP.subtract, op1=OP.max)
        nc.vector.tensor_tensor(P[:, 0:NV], ta[:, 0:NV], fo[:, 0:NV], op=OP.min)
        if not MASK_ONES:
            # P = relu(P + (mask - 1)); mask_g already holds (mask - 1)
            nc.vector.tensor_tensor(P[:], P[:], mask_g[g][0:GN, :], op=OP.add)
            nc.vector.tensor_scalar(P[:], P[:], 0.0, None, op0=OP.max)
        if dumps:
            nc.sync.dma_start(dumps["P_dump"][r0:r0 + GN, :], P[:])
        # straight-through boundaries: b = bs + (bh - bs)
        nc.scalar.activation(ta[:], P[:], AF.Tanh, scale=10.0)
        nc.scalar.activation(tb[:], P[:], AF.Tanh, scale=100000.0)
        nc.vector.tensor_tensor(tb[:], tb[:], ta[:], op=OP.subtract)
        nc.vector.tensor_tensor(ta[:], ta[:], tb[:], op=OP.add)
        if dumps:
            nc.sync.dma_start(dumps["b_dump"][r0:r0 + GN, :], ta[:])
        # cumsum along i, then +1 where first element == 0
        cc = cpool.tile([3, N], F32, name=f"cc_{g}", tag="fo")[0:GN]
        nc.vector.tensor_tensor_scan(cc[:], ta[:], zeros_bc[0:GN, :], 0.0,
                                     op0=OP.add, op1=OP.add)
        ind0 = cpool.tile([3, 1], F32, name=f"ind0_{g}", tag="ind0")[0:GN]
        nc.vector.tensor_scalar(ind0[:], cc[:, 0:1], 0.0, None, op0=OP.is_equal)
        nc.vector.tensor_scalar(cc[:], cc[:], ind0[:], None, op0=OP.add)
        if dumps:
            nc.sync.dma_start(dumps["c_dump"][r0:r0 + GN, :], cc[:])
        # c_last values gathered to partition 0 for runtime If conditions
        nc.sync.dma_start(clast_row[0:1, r0:r0 + GN],
                          cc[:, N - 1:N])
        cc_holder[g] = cc

    def phase_c_ct(g):
        # transpose c into ct (PE work, emitted late so it doesn't head-block
        # the PE queue while the DVE chain runs)
        GN = len(GROUPS[g])
        r0 = GROUPS[g][0]
        cc = cc_holder[g]
        for t in range(NT):
            psc = psA.tile([128, 4], F32, name="psc", tag="pst")[:, 0:GN]
            nc.tensor.transpose(psc[:], cc[:, t * 128:(t + 1) * 128],
                                ident4[0:GN, 0:GN])
            nc.vector.tensor_copy(
                ct[:, t * RPC + r0:t * RPC + r0 + GN], psc[:])

    # ------------- stage 3/4 per row: Wseg, pooling, MLP, store -------------
    def stage3(r):
        x16 = [x16_p.tile([128, H], F16, name=f"x16_{r}_{t}", tag=f"xr{t}")
               for t in range(NT)]
        for t in range(NT):
            xs = xn_p.tile([128, H], F32, name=f"xs_{r}_{t}", tag=f"xn{t}")
            nc.sync.dma_start(xs[:], x_d[r, t * 128:(t + 1) * 128, :])
            nc.gpsimd.tensor_copy(x16[t][:], xs[:])
        wr = [big_a.tile([128, 512], F16, name=f"wr_{r}_{k}", tag=f"bg{k}")
              for k in range(HT)]
        fac = {}   # per 128-word tile: [128,1] f32 column of 1/count

        def chunk(c, m0, m1):
            # sg/pool/cnt for word columns [c*512+m0, c*512+m1); pooled sums
            # land in wr[:, 0:m1-m0], counts in fac[global_mt] columns
            w = m1 - m0
            sgs = []
            for t in range(NT):
                ut = scr.tile([128, 512], F32, name="ut", tag="ut")
                nc.vector.tensor_scalar(ut[:, 0:w],
                                        iota_bc[:, c * 512 + m0:c * 512 + m1],
                                        ct[:, t * RPC + r:t * RPC + r + 1], None,
                                        op0=OP.subtract)
                nc.scalar.activation(ut[:, 0:w], ut[:, 0:w], AF.Abs)
                sg = sg_p.tile([128, 512], F16, name=f"sg_{t}", tag=f"sg{t}")
                # XLA f32 tanh saturates to 1.0 at |x| >= 7.90531110763549805,
                # which is what decides membership in the reference; the window
                # value itself cancels in the column normalization.
                nc.vector.tensor_scalar(sg[:, 0:w], ut[:, 0:w],
                                        7.90531110763549805e-5,
                                        None, op0=OP.is_lt)
                sgs.append(sg)
            for hh in range(HT):
                psp = psB.tile([128, 512], F32, name="psp", tag="mm")
                for t in range(NT):
                    nc.tensor.matmul(psp[:, 0:w],
                                     x16[t][:, hh * 128:(hh + 1) * 128],
                                     sgs[t][:, 0:w], start=(t == 0),
                                     stop=(t == NT - 1))
                nc.scalar.activation(wr[hh][:, 0:w], psp[:, 0:w], AF.Copy)
            # per-word counts as COLUMNS: sg slice stationary x ones moving ->
            # [128(m), 1] in PSUM; 1/count becomes a per-partition ACT scale
            # for mlp2 (no cross-partition broadcast needed)
            for ml in range(w // 128):
                gmt = (c * 512 + m0) // 128 + ml
                psC = psA.tile([128, 512], F32, name=f"psC_{gmt}", tag="pst")
                for t in range(NT):
                    nc.tensor.matmul(psC[:, 0:1],
                                     sgs[t][:, ml * 128:(ml + 1) * 128],
                                     ones16[:], start=(t == 0),
                                     stop=(t == NT - 1))
                fc = tiny.tile([128, 1], F32, name=f"fac_{r}_{gmt}",
                               tag=f"fac{gmt % 4}")
                nc.vector.tensor_scalar(fc[:], psC[:, 0:1], 1e-30, None,
                                        op0=OP.max)
                nc.vector.reciprocal(fc[:], fc[:])
                fac[gmt] = fc

        chunk(0, 0, MS)

        import numpy as _np
        engs = [mybir.EngineType.PE, mybir.EngineType.DVE,
                mybir.EngineType.Activation, mybir.EngineType.SP]
        creg = nc.alloc_registers(f"clast_{r}", engs)
        nc.regs_load(creg, clast_row[0:1, r:r + 1].bitcast(mybir.dt.int32))
        cval = nc.snap(creg, donate=True)
        creg2 = nc.alloc_registers(f"clastb_{r}", engs)
        nc.regs_load(creg2, clast_row[0:1, r:r + 1].bitcast(mybir.dt.int32))
        cval2 = nc.snap(creg2, donate=True)
        thr384 = int(_np.float32(float(MS) - 0.5).view(_np.int32))
        thr512 = int(_np.float32(511.5).view(_np.int32))

        r1m = [big_b.tile([128, 512], F16, name=f"r1m_{r}_{j}", tag=f"bb{j}")
               for j in range(HT)]

        # MLP layer 1: r1m[:, m0:m1] = relu(We1^T wr + be1)  (unnormalized;
        # the 1/count factor is applied per-partition in mlp2)
        def mlp1(m0, m1, w0):
            w = m1 - m0
            for j in range(HT):
                psm = psB.tile([128, 512], F32, name="psm", tag="mm")
                for k in range(HT):
                    nc.tensor.matmul(psm[:, 0:w],
                                     We1h[k][:, j * 128:(j + 1) * 128],
                                     wr[k][:, w0:w0 + w],
                                     start=(k == 0), stop=(k == HT - 1))
                nc.scalar.activation(r1m[j][:, m0:m1], psm[:, 0:w], AF.Relu,
                                     bias=be1c[:, j:j + 1])

        # MLP layer 2: out(m, h) = (r1m.T @ We2) * fac[m] (+ be2 if nonzero)
        def mlp2(mt):
            off = (mt % 4) * 128
            pso = psB.tile([128, 512], F32, name="pso", tag="mm")
            for j in range(HT):
                nc.tensor.matmul(pso[:], r1m[j][:, off:off + 128],
                                 We2h[j][:], start=(j == 0), stop=(j == HT - 1))
            ot = outp.tile([128, H], F32, name="ot", tag="ot")
            nc.scalar.activation(ot[:], pso[:], AF.Copy, scale=fac[mt][:])
            if not BIAS0:
                nc.vector.tensor_tensor(ot[:], ot[:], be2_bc[:], op=OP.add)
            nc.sync.dma_start(out_d[r, mt * 128:(mt + 1) * 128, :], ot[:])

        mlp1(0, MS, 0)
        for mt in range(MS // 128):
            mlp2(mt)
        if not SIM_SKIP:
            with tc.If(cval2 >= thr384):
                # words [MS, 512): redo sg/pool/cnt for that column range;
                # wr holds them at cols [0, 512-MS)
                chunk(0, MS, 512)
                mlp1(MS, 512, 0)
                mlp2(3)
            with tc.If(cval >= thr512):
                # words [512, 1024): full second chunk
                chunk(1, 0, 512)
                mlp1(0, 512, 0)
                for mt in range(4, NT):
                    mlp2(mt)

    # interleave: C(g0) overlaps stage1 of rows 2-3; C(g1) overlaps stage3 of
    # rows 0-1 — the PE never waits on a full-batch phase-C barrier.
    stage1(0)
    stage1(1)
    stage1(2)
    phase_c(0)
    stage1(3, pre_w2=lambda: phase_c_ct(0))
    stage3(0)
    stage3(1)
    stage3(2)
    phase_c(1)
    phase_c_ct(1)
    stage3(3)
    ctx.close()


def _get_module():
    key = ("nc", MASK_ONES, BIAS0)
    if key not in _cached:
        _cached[key] = _build_module()
    return _cached[key]


def _make_in_maps(inputs):
    x = np.ascontiguousarray(np.asarray(inputs["segment_rep"], dtype=np.float32))
    mask = np.ascontiguousarray(np.asarray(inputs["phn_mask"], dtype=np.float32))
    shared = {k: np.ascontiguousarray(np.asarray(inputs[k], np.float32))
              for k in ("W1", "b1", "W2", "We1", "be1", "We2", "be2")}
    shared["iota1024"] = np.arange(1, N + 1, dtype=np.float32).reshape(1, N)
    shared["iota128"] = np.arange(128, dtype=np.float32).reshape(1, 128)
    shared["idx128"] = np.arange(128, dtype=np.float32).reshape(128, 1)
    in_maps = []
    for core in range(NCORES):
        m = dict(shared)
        m["x"] = x[core * RPC:(core + 1) * RPC]
        m["mask"] = mask[core * RPC:(core + 1) * RPC]
        in_maps.append(m)
    return in_maps


def run_raw(inputs):
    """Run the SPMD kernel; returns list of per-core result dicts."""
    nc = _get_module()
    in_maps = _make_in_maps(inputs)
    res = run_bass_kernel_spmd(nc, in_maps, list(range(NCORES)))
    return res.results


def kernel(**inputs) -> np.ndarray:
    global MASK_ONES, BIAS0
    MASK_ONES = bool(np.all(np.asarray(inputs["phn_mask"]) == 1.0))
    BIAS0 = bool(np.all(np.asarray(inputs["be2"]) == 0.0))
    results = run_raw(inputs)
    out = np.concatenate([r["out"] for r in results], axis=0)
    return out.astype(np.float32)
